# revision 1
# baseline (speedup 1.0000x reference)
"""Trainium2 Bass kernel for nn_DAriEL_Decoder_Cell_1_88064009437441.

Key structural fact about the reference: the decoder cell resets
`one_softmax`/`unfolding` to their initial values at every t>0 (faithful
tf.cond port), so token selection at step t uses the UNIFORM distribution
and input_point[:, t] only — tokens never depend on the LM. The LM outputs
(the actual kernel result) are softmax(h_t) of a single 8-step LSTM scan
over the decoded tokens, since the per-step prefixes are nested.

Host: exact uniform-interval token search (all quantities are dyadic
rationals, exact in fp32) + embedding gather + weight re-layout (bf16).
Device (8 cores, SPMD): gate-dim (hidden-unit) sharded LSTM, 256 units
per core. x@Wk for all 8 steps is computed once up front; per step each
core seeds a PSUM bank with its zx slice (identity matmul) and
accumulates 16 bf16 h-ktile matmuls on top. Gates use tanh-only math
(sigmoid(x) = (tanh(x/2)+1)/2) so the scalar engine never swaps
activation tables except for the per-step Exp. The h slice is cast to
bf16, transposed on DVE (32x32 StreamTranspose), and AllGathered
together with the per-shard exp-sum so every core has the full
transposed h for the next step's matmul and the global softmax
denominator for its own output slice.
"""

import numpy as np

VOCAB = 2048
EMB = 256
MAXLEN = 8
BATCH = 16
NCORES = 8
U = VOCAB // NCORES          # 256 hidden units per core
AGW = 257 * 16               # allgather payload elems per core (4112)

_CACHE = {}


def _host_tokens(input_point):
    """token[b,t] = argmax_k((k/V <= v) & (v <= (k+1)/V)), first-true wins.
    Exact: v is fp32, k/V dyadic; replicate in float64."""
    v = input_point[:, :MAXLEN].astype(np.float64)
    u = v * VOCAB
    j = np.floor(u)
    exact = (u == j) & (j > 0)
    tok = np.where(exact, j - 1, j)
    return np.clip(tok, 0, VOCAB - 1).astype(np.int32)


def _build_nc(masked, b_nonzero):
    import concourse.bass as bass
    import concourse.mybir as mybir
    import concourse.tile as tile
    from concourse import bacc
    from concourse.masks import make_identity

    f32 = mybir.dt.float32
    bf16 = mybir.dt.bfloat16
    AF = mybir.ActivationFunctionType
    OP = mybir.AluOpType

    nc = bacc.Bacc(None, target_bir_lowering=False, debug=False)

    wblk = nc.dram_tensor("wblk", [2305, 4 * U], bf16, kind="ExternalInput")
    xt_ext = nc.dram_tensor("xt", [128, 2 * MAXLEN * BATCH], bf16, kind="ExternalInput")
    mask_ext = nc.dram_tensor("masks", [BATCH, MAXLEN], f32, kind="ExternalInput")
    out_ext = nc.dram_tensor("out", [BATCH, MAXLEN * U], f32, kind="ExternalOutput")

    from contextlib import ExitStack

    with ExitStack() as _st:
        rsems = [
            _st.enter_context(nc.semaphore(f"rsem{t}")) for t in range(MAXLEN)
        ]
        lsem = _st.enter_context(nc.semaphore("lsem"))
        tc = _st.enter_context(tile.TileContext(nc))
        with (
            tc.tile_pool(name="const", bufs=1) as cp,
            tc.tile_pool(name="state", bufs=2) as sp,
            tc.tile_pool(name="work", bufs=3) as wk,
            tc.tile_pool(name="zps", bufs=2, space="PSUM") as psz,
            tc.tile_pool(name="zxps", bufs=1, space="PSUM") as psx,
        ):
            # semaphores persist across NEFF runs: clear ours, then barrier so
            # no core sends before every core has cleared
            with tc.tile_critical():
                for s_ in rsems:
                    nc.gpsimd.sem_clear(s_)
                nc.gpsimd.sem_clear(lsem)
                nc.gpsimd.bir_kernel_barrier_wait([list(range(NCORES))])

            identity = cp.tile([128, 128], f32)
            make_identity(nc, identity[:])
            idb = cp.tile([16, 16], bf16)
            nc.vector.tensor_copy(idb[:], identity[0:16, 0:16])

            xt_sb = cp.tile([128, 2 * MAXLEN * BATCH], bf16)
            nc.sync.dma_start(xt_sb[:], xt_ext[:])
            mask_sb = cp.tile([BATCH, MAXLEN], f32)
            nc.sync.dma_start(mask_sb[:], mask_ext[:])

            wsb = cp.tile([128, 18 * 1024], bf16)
            # x-tiles first (zx precompute unblocks), then h-tiles;
            # round-robin issue engines so descriptor gen parallelizes
            dma_engs = [nc.sync, nc.scalar, nc.gpsimd]
            for i, j in enumerate((16, 17)):
                dma_engs[i % 3].dma_start(
                    wsb[:, 1024 * j:1024 * (j + 1)], wblk[128 * j:128 * (j + 1), :]
                )
            if b_nonzero:
                wb = cp.tile([1, 1024], bf16)
                onesrow = cp.tile([1, 128], bf16)
                nc.gpsimd.memset(onesrow[:], 1.0)
                nc.sync.dma_start(wb[:], wblk[2304:2305, :])
            for j in range(16):
                dma_engs[j % 3].dma_start(
                    wsb[:, 1024 * j:1024 * (j + 1)], wblk[128 * j:128 * (j + 1), :]
                )

            # ---- zx = x @ Wk (+ b) for all steps at once: (128=(t,b), 1024) ----
            zx_ps0 = psx.tile([128, 512], f32, tag="zx0")
            zx_ps1 = psx.tile([128, 512], f32, tag="zx1")
            zx_ps = [zx_ps0, zx_ps1]
            for i, cb in ((0, 0), (1, 512)):
                nc.tensor.matmul(
                    zx_ps[i][:], xt_sb[:, 0:128],
                    wsb[:, 1024 * 16 + cb:1024 * 16 + cb + 512],
                    start=True, stop=False,
                )
                nc.tensor.matmul(
                    zx_ps[i][:], xt_sb[:, 128:256],
                    wsb[:, 1024 * 17 + cb:1024 * 17 + cb + 512],
                    start=False, stop=not b_nonzero,
                )
                if b_nonzero:
                    nc.tensor.matmul(
                        zx_ps[i][:], onesrow[:], wb[:, cb:cb + 512],
                        start=False, stop=True,
                    )
            zx_sb = cp.tile([128, 1024], bf16)
            nc.vector.tensor_copy(zx_sb[:, 0:512], zx_ps[0][:])
            nc.vector.tensor_copy(zx_sb[:, 512:1024], zx_ps[1][:])
            # rearrange to (16=batch, 8 steps x 1024)
            zx_steps = cp.tile([BATCH, MAXLEN * 1024], bf16)
            for t in range(MAXLEN):
                nc.sync.dma_start(
                    zx_steps[:, 1024 * t:1024 * (t + 1)],
                    zx_sb[16 * t:16 * (t + 1), :],
                )

            exp_sb = cp.tile([BATCH, MAXLEN * U], f32)
            out_sb = cp.tile([BATCH, MAXLEN * U], f32)

            # bf16 h staging for transpose: rows 16:32 zeroed once per slot
            hb_slots = []
            for i in range(2):
                hb = cp.tile([32, U], bf16, tag=f"hb{i}")
                nc.vector.memset(hb[:], 0.0)
                hb_slots.append(hb)

            # remote-DMA gather buffers (static addresses, identical on all
            # cores). pay = [hT image (128x64, batch padded to 32) | sums col].
            # hT_slots[s][:, 65*d:65*(d+1)] receives source at XOR-position d.
            pay_slots = []
            hT_slots = []
            for i in range(2):
                pay = cp.tile([128, 65], bf16, tag=f"pay{i}")
                nc.vector.memset(pay[:], 0.0)
                pay_slots.append(pay)
                hTs = cp.tile([128, 8 * 65], bf16, tag=f"hTs{i}")
                hT_slots.append(hTs)
            # send #j of each step: dest = (0, j) relative (XOR), lands at
            # receiver position j; other 7 slots dummy
            rdests_all = [
                [(0, j) if i == j else None for i in range(8)] for j in range(8)
            ]

            if masked:
                h_prev = sp.tile([BATCH, U], f32, tag="h")
                nc.vector.memset(h_prev[:], 0.0)
            c_prev = sp.tile([BATCH, U], f32, tag="c")
            nc.vector.memset(c_prev[:], 0.0)
            hT_full = None

            def softmax_out(t):
                """Global denominator from gathered per-shard sums, then scale
                own exp slice and DMA out. Requires rsem >= 16*(t+1)."""
                hTs = hT_slots[t % 2]
                sums_ap = hTs[0:16, :].rearrange("b (r c) -> b r c", c=65)[:, :, 64]
                tot = wk.tile([BATCH, 1], f32, tag="tot")
                rec = wk.tile([BATCH, 1], f32, tag="rec")
                nc.vector.tensor_reduce(
                    tot[:], sums_ap, mybir.AxisListType.X, OP.add
                )
                nc.vector.reciprocal(rec[:], tot[:])
                nc.vector.tensor_scalar_mul(
                    out_sb[:, U * t:U * (t + 1)], exp_sb[:, U * t:U * (t + 1)],
                    rec[:],
                )
                nc.gpsimd.dma_start(
                    out_ext[:, U * t:U * (t + 1)], out_sb[:, U * t:U * (t + 1)]
                )

            for t in range(MAXLEN):
                # ---- wait for step t-1's gather; fake-write the slot so Tile
                # sees a producer for everything that reads it ----
                if t > 0:
                    hTs = hT_slots[(t - 1) % 2]
                    with tc.tile_critical():
                        nc.vector.wait_ge(rsems[t - 1], 14)
                        # own sends of steps < t have fully drained (releases
                        # the pay slot this step will overwrite)
                        nc.vector.wait_ge(lsem, 112 * t)
                        nc.vector.tensor_copy(hTs[0:1, 0:1], hTs[0:1, 0:1])
                    softmax_out(t - 1)

                # ---- z for step t in 2 PSUM banks (or zx only at t=0) ----
                if t == 0:
                    z0 = zx_steps[:, 0:512]
                    z1 = zx_steps[:, 512:1024]
                else:
                    pz0 = psz.tile([BATCH, 512], f32, tag="z0")
                    pz1 = psz.tile([BATCH, 512], f32, tag="z1")
                    for pz, cb in ((pz0, 0), (pz1, 512)):
                        nc.tensor.matmul(
                            pz[:], idb[:],
                            zx_steps[:, 1024 * t + cb:1024 * t + cb + 512],
                            start=True, stop=False,
                        )
                        for j in range(16):
                            d, m = j // 2, j % 2
                            nc.tensor.matmul(
                                pz[:], hTs[:, 65 * d + 32 * m:65 * d + 32 * m + 16],
                                wsb[:, 1024 * j + cb:1024 * j + cb + 512],
                                start=False, stop=(j == 15),
                            )
                    z0, z1 = pz0[:], pz1[:]

                # ---- gates, tanh-only: sig(x) = (tanh(x/2)+1)/2 ----
                # z layout: [i (0:256) | f (256:512)] in z0, [g | o] in z1
                t_if = wk.tile([BATCH, 2 * U], f32, tag="tif")
                tg = wk.tile([BATCH, U], f32, tag="tg")
                t_o = wk.tile([BATCH, U], f32, tag="to")
                nc.scalar.activation(t_if[:], z0, AF.Tanh, scale=0.5)
                nc.scalar.activation(tg[:], z1[:, 0:U], AF.Tanh)
                nc.scalar.activation(t_o[:], z1[:, U:2 * U], AF.Tanh, scale=0.5)

                hbt = hb_slots[t % 2]
                acc = wk.tile([BATCH, 1], f32, tag="acc")
                c_next = sp.tile([BATCH, U], f32, tag="c")
                if not masked:
                    # State C = 2c, hb = 2h; Wr is pre-halved on the host so
                    # z += (2h) @ (Wr/2). 2*sig(x) = tanh(x/2)+1.
                    # C_new = 0.5*(t_f+1)*C + (t_i+1)*tg
                    t1 = wk.tile([BATCH, U], f32, tag="t1")
                    t2 = wk.tile([BATCH, U], f32, tag="t2")
                    nc.vector.scalar_tensor_tensor(
                        t2[:], t_if[:, 0:U], 1.0, tg[:], OP.add, OP.mult
                    )
                    nc.vector.scalar_tensor_tensor(
                        t1[:], t_if[:, U:2 * U], 1.0, c_prev[:], OP.add, OP.mult
                    )
                    nc.vector.scalar_tensor_tensor(
                        c_next[:], t1[:], 0.5, t2[:], OP.mult, OP.add
                    )
                    tc_ = wk.tile([BATCH, U], f32, tag="tc")
                    nc.scalar.activation(tc_[:], c_next[:], AF.Tanh, scale=0.5)
                    # hb = 2h = (t_o+1)*tanh(c), written bf16 directly
                    nc.vector.scalar_tensor_tensor(
                        hbt[0:16, :], t_o[:], 1.0, tc_[:], OP.add, OP.mult
                    )
                    # exp(h) = exp(hb * 0.5), numerator + per-shard sum
                    nc.scalar.activation(
                        exp_sb[:, U * t:U * (t + 1)], hbt[0:16, :], AF.Exp,
                        scale=0.5, accum_out=acc[:],
                    )
                else:
                    # Same doubled representation (C=2c, H=2h); mask blending
                    # is linear so it works unchanged on doubled state.
                    m_t = mask_sb[:, t:t + 1]
                    t1 = wk.tile([BATCH, U], f32, tag="t1")
                    t2 = wk.tile([BATCH, U], f32, tag="t2")
                    cn = wk.tile([BATCH, U], f32, tag="cn")
                    dm = wk.tile([BATCH, U], f32, tag="dm")
                    nc.vector.scalar_tensor_tensor(
                        t2[:], t_if[:, 0:U], 1.0, tg[:], OP.add, OP.mult
                    )
                    nc.vector.scalar_tensor_tensor(
                        t1[:], t_if[:, U:2 * U], 1.0, c_prev[:], OP.add, OP.mult
                    )
                    nc.vector.scalar_tensor_tensor(
                        cn[:], t1[:], 0.5, t2[:], OP.mult, OP.add
                    )
                    nc.vector.tensor_tensor(dm[:], cn[:], c_prev[:], OP.subtract)
                    nc.vector.scalar_tensor_tensor(
                        c_next[:], dm[:], m_t, c_prev[:], OP.mult, OP.add
                    )
                    tc_ = wk.tile([BATCH, U], f32, tag="tc")
                    nc.scalar.activation(tc_[:], c_next[:], AF.Tanh, scale=0.5)
                    hn = wk.tile([BATCH, U], f32, tag="hn")
                    dh = wk.tile([BATCH, U], f32, tag="dh")
                    h_next = sp.tile([BATCH, U], f32, tag="h")
                    nc.vector.scalar_tensor_tensor(
                        hn[:], t_o[:], 1.0, tc_[:], OP.add, OP.mult
                    )
                    nc.vector.tensor_tensor(dh[:], hn[:], h_prev[:], OP.subtract)
                    nc.vector.scalar_tensor_tensor(
                        h_next[:], dh[:], m_t, h_prev[:], OP.mult, OP.add
                    )
                    nc.vector.tensor_copy(hbt[0:16, :], h_next[:])
                    nc.scalar.activation(
                        exp_sb[:, U * t:U * (t + 1)], h_next[:], AF.Exp,
                        scale=0.5, accum_out=acc[:],
                    )
                    h_prev = h_next

                # ---- build payload: transpose 2h into hT image + sums col ----
                pay = pay_slots[t % 2]
                for q in range(8):
                    ph, m = q % 4, q // 4
                    nc.vector.transpose(
                        pay[32 * ph:32 * ph + 32, 32 * m:32 * m + 32],
                        hbt[:, 32 * q:32 * q + 32],
                    )
                nc.vector.tensor_copy(pay[0:16, 64:65], acc[:])

                # ---- push slice to peers' hT slots via remote DMA; own
                # slice (XOR position 0) is a local copy ----
                nc.vector.tensor_copy(
                    hT_slots[t % 2][:, 0:65], pay[:, 0:65]
                )
                for j in range(1, NCORES):
                    nc.gpsimd.remote_dma_broadcast(
                        hT_slots[t % 2][:, 65 * j:65 * (j + 1)],
                        pay[:, 0:65],
                        rsems[t],
                        lsem,
                        rdests=rdests_all[j],
                    )
                nc.gpsimd.trigger_dma(count=None)

                c_prev = c_next

            # ---- last step's softmax needs the final gather ----
            hTs = hT_slots[(MAXLEN - 1) % 2]
            with tc.tile_critical():
                nc.vector.wait_ge(rsems[MAXLEN - 1], 14)
                nc.vector.tensor_copy(hTs[0:1, 0:1], hTs[0:1, 0:1])
            softmax_out(MAXLEN - 1)

    nc.compile()
    return nc


def _get_nc(masked=False, b_nonzero=False):
    key = (masked, b_nonzero)
    if key not in _CACHE:
        _CACHE[key] = _build_nc(masked, b_nonzero)
    return _CACHE[key]


def _host_prep(input_point, E, Wk, Wr, b, hw_swizzle=True):
    """hw_swizzle: the real ucode's D2D lanes (slots 4-7) land data at the
    XOR position with an extra ^2 (measured by probe.py); CoreSim models the
    pure-XOR placement."""
    def src(c, d):
        return c ^ d ^ (2 if (d & 4) and hw_swizzle else 0)

    import ml_dtypes
    bf = ml_dtypes.bfloat16

    ip = np.ascontiguousarray(np.asarray(input_point, dtype=np.float32))
    E = np.asarray(E, dtype=np.float32)
    Wk = np.asarray(Wk, dtype=np.float32)
    Wr = np.asarray(Wr, dtype=np.float32)
    b = np.asarray(b, dtype=np.float32)

    tokens = _host_tokens(ip)                                # (B, T)
    masks = (tokens != 0).astype(np.float32)                 # (B, T)
    X = E[tokens]                                            # (B, T, EMB)

    # xt[p, 128*half + 16*t + b] = X[b, t, 128*half + p]
    xt = np.transpose(X.reshape(BATCH, MAXLEN, 2, 128), (2, 3, 1, 0))  # (2,128,T,B)
    xt = np.ascontiguousarray(
        np.transpose(xt, (1, 0, 2, 3)).reshape(128, 2 * MAXLEN * BATCH)
    ).astype(bf)

    # Wr halved: the device carries h in doubled representation (hb = 2h).
    # Per-core XOR row permutation: gathered position d holds source core
    # (d XOR k), so core k's Wr row-blocks are ordered [256*(d^k)] for d=0..7.
    Wr2 = Wr * 0.5
    tail = np.vstack([Wk, b[None, :]]).astype(np.float32)    # (257, 4V)
    in_maps = []
    for k in range(NCORES):
        cols = np.concatenate(
            [np.arange(g * VOCAB + k * U, g * VOCAB + (k + 1) * U) for g in range(4)]
        )
        rows = np.concatenate(
            [np.arange(256 * src(k, d), 256 * (src(k, d) + 1)) for d in range(NCORES)]
        )
        W_aug = np.vstack([Wr2[rows], tail])                 # (2305, 4V)
        in_maps.append({
            "wblk": np.ascontiguousarray(W_aug[:, cols]).astype(bf),
            "xt": xt,
            "masks": np.ascontiguousarray(masks),
        })
    flags = (bool((masks != 1.0).any()), bool(np.any(b != 0.0)))
    return in_maps, flags


def kernel(input_point, E, Wk, Wr, b):
    from concourse.bass_utils import run_bass_kernel_spmd

    in_maps, flags = _host_prep(input_point, E, Wk, Wr, b)
    nc = _get_nc(*flags)
    res = run_bass_kernel_spmd(nc, in_maps, list(range(NCORES)))
    results = res.results

    out = np.empty((BATCH, MAXLEN, VOCAB), dtype=np.float32)
    for k in range(NCORES):
        blk = results[k]["out"].reshape(BATCH, MAXLEN, U)    # (B, T, U)
        out[:, :, k * U:(k + 1) * U] = blk
    return out



# revision 7
# speedup vs baseline: 1.7071x; 1.7071x over previous
"""Trainium2 Bass kernel for nn_DAriEL_Decoder_Cell_1_88064009437441.

Reference structure: tokens depend only on input_point (the cell resets
one_softmax/unfolding each step), so the device work is one 8-step LSTM
(2048 units, gate dim 8192) + per-step softmax of h_t.

Distribution: 4-way tensor-parallel over the hidden/gate dim (U=512 per
shard), with each shard REPLICATED on both dies (core r holds shard r&3).
Every core therefore gathers the other 3 h-slices from SAME-DIE peers
(r^1, r^2, r^3) via repeated-dest relative remote_dma_broadcast — all 16
DMA lanes carry real payload (the per-peer single-dest broadcast of the
old kernel serialized ~42us/step of dummy-lane descriptors; this one does
~2.3us/call with pure-XOR placement, probe-verified).

x@Wk for all gates rides in the same PSUM accumulation as two extra
k-tiles (stationary = per-step x^T slice), so there is no zx precompute
and no identity-seed matmuls. Next step's x-tile matmuls are emitted
right after this step's h-tiles to shorten PE idle gaps (HAM warmth).
"""

import numpy as np

VOCAB = 2048
EMB = 256
MAXLEN = 8
BATCH = 16
NCORES = 8
NSHARD = 4
U = VOCAB // NSHARD          # 512 hidden units per core (shard = rank & 3)
PCOLS = 4 * 32 + 1           # pay cols: hT image 128 + sums col = 129

_CACHE = {}


def _host_tokens(input_point):
    """token[b,t] = argmax_k((k/V <= v) & (v <= (k+1)/V)), first-true wins.
    Exact: v is fp32, k/V dyadic; replicate in float64."""
    v = input_point[:, :MAXLEN].astype(np.float64)
    u = v * VOCAB
    j = np.floor(u)
    exact = (u == j) & (j > 0)
    tok = np.where(exact, j - 1, j)
    return np.clip(tok, 0, VOCAB - 1).astype(np.int32)


def _build_nc(masked, b_nonzero):
    import concourse.mybir as mybir
    import concourse.tile as tile
    from concourse import bacc

    f32 = mybir.dt.float32
    bf16 = mybir.dt.bfloat16
    AF = mybir.ActivationFunctionType
    OP = mybir.AluOpType

    nc = bacc.Bacc(None, target_bir_lowering=False, debug=False)

    # wblk rows: 16 h-ktiles (XOR-permuted Wr/2) + 2 x-ktiles (Wk) + bias
    wblk = nc.dram_tensor("wblk", [2305, 4 * U], bf16, kind="ExternalInput")
    xt_ext = nc.dram_tensor("xt", [128, 2 * MAXLEN * BATCH], bf16, kind="ExternalInput")
    mask_ext = nc.dram_tensor("masks", [BATCH, MAXLEN], f32, kind="ExternalInput")
    out_ext = nc.dram_tensor("out", [BATCH, MAXLEN * U], f32, kind="ExternalOutput")

    from contextlib import ExitStack

    with ExitStack() as _st:
        rsem = _st.enter_context(nc.semaphore("rsem"))
        lsem = _st.enter_context(nc.semaphore("lsem"))
        tc = _st.enter_context(tile.TileContext(nc))
        with (
            tc.tile_pool(name="const", bufs=1) as cp,
            tc.tile_pool(name="state", bufs=2) as sp,
            tc.tile_pool(name="work", bufs=3) as wk,
            tc.tile_pool(name="zps", bufs=2, space="PSUM") as psz,
        ):
            with tc.tile_critical():
                nc.gpsimd.sem_clear(rsem)
                nc.gpsimd.sem_clear(lsem)
                nc.gpsimd.bir_kernel_barrier_wait([list(range(NCORES))])

            xt_sb = cp.tile([128, 2 * MAXLEN * BATCH], bf16)
            nc.sync.dma_start(xt_sb[:], xt_ext[:])
            mask_sb = cp.tile([BATCH, MAXLEN], f32)
            nc.sync.dma_start(mask_sb[:], mask_ext[:])

            W = 4 * U  # 2048 cols per core: [i|f|g|o] of shard b
            wsb = cp.tile([128, 18 * W], bf16)
            dma_engs = [nc.sync, nc.scalar, nc.gpsimd]
            for i, j in enumerate((16, 17)):  # x-tiles first
                dma_engs[i % 3].dma_start(
                    wsb[:, W * j:W * (j + 1)], wblk[128 * j:128 * (j + 1), :]
                )
            if b_nonzero:
                wb = cp.tile([1, W], bf16)
                onesrow = cp.tile([1, 16], bf16)
                nc.gpsimd.memset(onesrow[:], 1.0)
                nc.sync.dma_start(wb[:], wblk[2304:2305, :])
            for j in range(16):
                dma_engs[j % 3].dma_start(
                    wsb[:, W * j:W * (j + 1)], wblk[128 * j:128 * (j + 1), :]
                )

            exp_sb = cp.tile([BATCH, MAXLEN * U], f32)
            out_sb = cp.tile([BATCH, MAXLEN * U], f32)

            # bf16 h staging for DVE transpose: rows 16:32 zeroed once
            hb_slots = []
            for i in range(2):
                hb = cp.tile([32, U], bf16, tag=f"hb{i}")
                nc.vector.memset(hb[:], 0.0)
                hb_slots.append(hb)

            # pay = [hT image (128 x 128, batch padded to 32) | sums col]
            pay_slots = []
            hT_slots = []
            for i in range(2):
                pay = cp.tile([128, PCOLS], bf16, tag=f"pay{i}")
                nc.vector.memset(pay[:], 0.0)
                pay_slots.append(pay)
                hTs = cp.tile([128, 4 * PCOLS], bf16, tag=f"hTs{i}")
                hT_slots.append(hTs)
            rd_same = {d: [(0, d)] * 8 for d in (1, 2, 3)}

            if masked:
                h_prev = sp.tile([BATCH, U], f32, tag="h")
                nc.vector.memset(h_prev[:], 0.0)
            c_prev = sp.tile([BATCH, U], f32, tag="c")
            nc.vector.memset(c_prev[:], 0.0)

            def stat_ap(t, d, m):
                """stationary [128,16] for k-tile (d,m) of step t's h."""
                if d == 0:
                    return pay_slots[t % 2][:, 32 * m:32 * m + 16]
                return hT_slots[t % 2][:, PCOLS * d + 32 * m:PCOLS * d + 32 * m + 16]

            def softmax_out(t):
                """Denominator from own acc + 3 gathered sums; scale + out."""
                hTs = hT_slots[t % 2]
                sums_ap = hTs[0:16, :].rearrange(
                    "b (d c) -> b d c", c=PCOLS
                )[:, 1:4, PCOLS - 1]
                tot3 = wk.tile([BATCH, 1], f32, tag="tot")
                tot = wk.tile([BATCH, 1], f32, tag="tot4")
                rec = wk.tile([BATCH, 1], f32, tag="rec")
                nc.vector.tensor_reduce(
                    tot3[:], sums_ap, mybir.AxisListType.X, OP.add
                )
                nc.vector.tensor_tensor(tot[:], tot3[:], accs[t][:], OP.add)
                nc.vector.reciprocal(rec[:], tot[:])
                nc.vector.tensor_scalar_mul(
                    out_sb[:, U * t:U * (t + 1)], exp_sb[:, U * t:U * (t + 1)],
                    rec[:],
                )
                nc.gpsimd.dma_start(
                    out_ext[:, U * t:U * (t + 1)], out_sb[:, U * t:U * (t + 1)]
                )

            accs = {}
            z_tiles = {}

            def x_rounds(t):
                """Seed step t's 4 z banks with x@Wk (+ b): 2 k-tiles."""
                zt = [
                    psz.tile([BATCH, 512], f32, tag=f"z{bk}", name=f"z{bk}")
                    for bk in range(4)
                ]
                z_tiles[t] = zt
                # step 0 has no h-rounds: its group closes here
                last = (t == 0)
                for kx in (16, 17):
                    xs = xt_sb[:, 16 * ((kx - 16) * MAXLEN + t):
                               16 * ((kx - 16) * MAXLEN + t) + 16]
                    for bk in range(4):
                        nc.tensor.matmul(
                            zt[bk][:], xs, wsb[:, W * kx + 512 * bk:W * kx + 512 * (bk + 1)],
                            start=(kx == 16),
                            stop=(last and kx == 17 and not b_nonzero),
                        )
                if b_nonzero:
                    for bk in range(4):
                        nc.tensor.matmul(
                            zt[bk][:], onesrow[:], wb[:, 512 * bk:512 * (bk + 1)],
                            start=False, stop=last,
                        )

            x_rounds(0)
            for t in range(MAXLEN):
                zt = z_tiles[t]
                # ---- h-rounds: slice-major, pipelined on arrivals ----
                # consumes step t-1's h: pay/hT slots of parity (t-1) % 2
                if t > 0:
                    hTs = hT_slots[(t - 1) % 2]
                    with tc.tile_critical():
                        nc.vector.wait_ge(rsem, 48 * t)
                        nc.vector.tensor_copy(hTs[0:1, 0:1], hTs[0:1, 0:1])
                    for d in range(4):
                        for m in range(4):
                            kt = 4 * d + m
                            st = stat_ap(t - 1, d, m)
                            for bk in range(4):
                                nc.tensor.matmul(
                                    zt[bk][:], st,
                                    wsb[:, W * kt + 512 * bk:W * kt + 512 * (bk + 1)],
                                    start=False, stop=(kt == 15),
                                )
                    # own sends of step t-2 drained (pay slot reuse)
                    with tc.tile_critical():
                        nc.vector.wait_ge(lsem, 48 * (t - 1))
                        nc.vector.tensor_copy(
                            pay_slots[t % 2][0:1, 0:1], pay_slots[t % 2][0:1, 0:1]
                        )
                    softmax_out(t - 1)
                if t + 1 < MAXLEN:
                    x_rounds(t + 1)

                # ---- gates, tanh-only: sig(x) = (tanh(x/2)+1)/2 ----
                t_i = wk.tile([BATCH, U], f32, tag="ti")
                t_f = wk.tile([BATCH, U], f32, tag="tf")
                tg = wk.tile([BATCH, U], f32, tag="tg")
                t_o = wk.tile([BATCH, U], f32, tag="to")
                nc.scalar.activation(t_i[:], zt[0][:], AF.Tanh, scale=0.5)
                nc.scalar.activation(t_f[:], zt[1][:], AF.Tanh, scale=0.5)
                nc.scalar.activation(tg[:], zt[2][:], AF.Tanh)
                nc.scalar.activation(t_o[:], zt[3][:], AF.Tanh, scale=0.5)

                hbt = hb_slots[t % 2]
                acc = wk.tile([BATCH, 1], f32, tag=f"acc{t}")
                accs[t] = acc
                c_next = sp.tile([BATCH, U], f32, tag="c")
                t1 = wk.tile([BATCH, U], f32, tag="t1")
                t2 = wk.tile([BATCH, U], f32, tag="t2")
                if not masked:
                    # doubled state: C = 2c, hb = 2h; Wr pre-halved on host
                    nc.vector.scalar_tensor_tensor(
                        t2[:], t_i[:], 1.0, tg[:], OP.add, OP.mult
                    )
                    nc.vector.scalar_tensor_tensor(
                        t1[:], t_f[:], 1.0, c_prev[:], OP.add, OP.mult
                    )
                    nc.vector.scalar_tensor_tensor(
                        c_next[:], t1[:], 0.5, t2[:], OP.mult, OP.add
                    )
                    tc_ = wk.tile([BATCH, U], f32, tag="tc")
                    nc.scalar.activation(tc_[:], c_next[:], AF.Tanh, scale=0.5)
                    nc.vector.scalar_tensor_tensor(
                        hbt[0:16, :], t_o[:], 1.0, tc_[:], OP.add, OP.mult
                    )
                    nc.scalar.activation(
                        exp_sb[:, U * t:U * (t + 1)], hbt[0:16, :], AF.Exp,
                        scale=0.5, accum_out=acc[:],
                    )
                else:
                    m_t = mask_sb[:, t:t + 1]
                    cn = wk.tile([BATCH, U], f32, tag="cn")
                    dm = wk.tile([BATCH, U], f32, tag="dm")
                    nc.vector.scalar_tensor_tensor(
                        t2[:], t_i[:], 1.0, tg[:], OP.add, OP.mult
                    )
                    nc.vector.scalar_tensor_tensor(
                        t1[:], t_f[:], 1.0, c_prev[:], OP.add, OP.mult
                    )
                    nc.vector.scalar_tensor_tensor(
                        cn[:], t1[:], 0.5, t2[:], OP.mult, OP.add
                    )
                    nc.vector.tensor_tensor(dm[:], cn[:], c_prev[:], OP.subtract)
                    nc.vector.scalar_tensor_tensor(
                        c_next[:], dm[:], m_t, c_prev[:], OP.mult, OP.add
                    )
                    tc_ = wk.tile([BATCH, U], f32, tag="tc")
                    nc.scalar.activation(tc_[:], c_next[:], AF.Tanh, scale=0.5)
                    hn = wk.tile([BATCH, U], f32, tag="hn")
                    dh = wk.tile([BATCH, U], f32, tag="dh")
                    h_next = sp.tile([BATCH, U], f32, tag="h")
                    nc.vector.scalar_tensor_tensor(
                        hn[:], t_o[:], 1.0, tc_[:], OP.add, OP.mult
                    )
                    nc.vector.tensor_tensor(dh[:], hn[:], h_prev[:], OP.subtract)
                    nc.vector.scalar_tensor_tensor(
                        h_next[:], dh[:], m_t, h_prev[:], OP.mult, OP.add
                    )
                    nc.vector.tensor_copy(hbt[0:16, :], h_next[:])
                    nc.scalar.activation(
                        exp_sb[:, U * t:U * (t + 1)], h_next[:], AF.Exp,
                        scale=0.5, accum_out=acc[:],
                    )
                    h_prev = h_next

                # ---- transpose 2h into pay hT image + sums col ----
                pay = pay_slots[t % 2]
                for q in range(16):
                    ph, m = q % 4, q // 4
                    nc.vector.transpose(
                        pay[32 * ph:32 * ph + 32, 32 * m:32 * m + 32],
                        hbt[:, 32 * q:32 * q + 32],
                    )
                nc.vector.tensor_copy(pay[0:16, PCOLS - 1:PCOLS], acc[:])

                # ---- 3 same-die repeated-dest broadcasts ----
                for d in (1, 2, 3):
                    nc.gpsimd.remote_dma_broadcast(
                        hT_slots[t % 2][:, PCOLS * d:PCOLS * (d + 1)],
                        pay[:, 0:PCOLS],
                        rsem,
                        lsem,
                        rdests=rd_same[d],
                    )
                nc.gpsimd.trigger_dma(count=None)

                c_prev = c_next

            # ---- last step's softmax needs the final gather ----
            hTs = hT_slots[(MAXLEN - 1) % 2]
            with tc.tile_critical():
                nc.vector.wait_ge(rsem, 48 * MAXLEN)
                nc.vector.tensor_copy(hTs[0:1, 0:1], hTs[0:1, 0:1])
            softmax_out(MAXLEN - 1)

    nc.compile()
    return nc


def _get_nc(masked=False, b_nonzero=False):
    key = (masked, b_nonzero)
    if key not in _CACHE:
        _CACHE[key] = _build_nc(masked, b_nonzero)
    return _CACHE[key]


def _host_prep(input_point, E, Wk, Wr, b):
    import ml_dtypes
    bf = ml_dtypes.bfloat16

    ip = np.ascontiguousarray(np.asarray(input_point, dtype=np.float32))
    E = np.asarray(E, dtype=np.float32)
    Wk = np.asarray(Wk, dtype=np.float32)
    Wr = np.asarray(Wr, dtype=np.float32)
    b = np.asarray(b, dtype=np.float32)

    tokens = _host_tokens(ip)                                # (B, T)
    masks = (tokens != 0).astype(np.float32)                 # (B, T)
    X = E[tokens]                                            # (B, T, EMB)

    # xt[p, 16*(kt*T + t) + bb] = X[bb, t, 128*kt + p]
    xt = np.transpose(X.reshape(BATCH, MAXLEN, 2, 128), (2, 3, 1, 0))  # (2,128,T,B)
    xt = np.ascontiguousarray(xt.reshape(2, 128, MAXLEN * BATCH))
    xt = np.concatenate([xt[0], xt[1]], axis=1).astype(bf)   # (128, 2*T*B)

    Wr2 = Wr * 0.5
    tail = np.vstack([Wk, b[None, :]]).astype(np.float32)    # (257, 4V)
    in_maps = []
    for r in range(NCORES):
        sh = r & 3
        cols = np.concatenate(
            [np.arange(g * VOCAB + sh * U, g * VOCAB + (sh + 1) * U)
             for g in range(4)]
        )
        rows = np.concatenate(
            [np.arange(U * (sh ^ d), U * ((sh ^ d) + 1)) for d in range(NSHARD)]
        )
        W_aug = np.vstack([Wr2[rows], tail])                 # (2305, 4V)
        in_maps.append({
            "wblk": np.ascontiguousarray(W_aug[:, cols]).astype(bf),
            "xt": xt,
            "masks": np.ascontiguousarray(masks),
        })
    flags = (bool((masks != 1.0).any()), bool(np.any(b != 0.0)))
    return in_maps, flags


def _assemble(results):
    out = np.empty((BATCH, MAXLEN, VOCAB), dtype=np.float32)
    for r in range(NSHARD):
        blk = results[r]["out"].reshape(BATCH, MAXLEN, U)
        out[:, :, r * U:(r + 1) * U] = blk
    return out


def kernel(input_point, E, Wk, Wr, b):
    from concourse.bass_utils import run_bass_kernel_spmd

    in_maps, flags = _host_prep(input_point, E, Wk, Wr, b)
    nc = _get_nc(*flags)
    res = run_bass_kernel_spmd(nc, in_maps, list(range(NCORES)))
    return _assemble(res.results)


# revision 10
# speedup vs baseline: 1.7810x; 1.0433x over previous
"""Trainium2 Bass kernel for nn_DAriEL_Decoder_Cell_1_88064009437441.

Reference structure: tokens depend only on input_point (the cell resets
one_softmax/unfolding each step), so the device work is one 8-step LSTM
(2048 units, gate dim 8192) + per-step softmax of h_t.

Distribution: 4-way tensor-parallel over the hidden/gate dim (U=512 per
shard), with each shard REPLICATED on both dies (core r holds shard r&3).
Every core therefore gathers the other 3 h-slices from SAME-DIE peers
(r^1, r^2, r^3) via repeated-dest relative remote_dma_broadcast — all 16
DMA lanes carry real payload (the per-peer single-dest broadcast of the
old kernel serialized ~42us/step of dummy-lane descriptors; this one does
~2.3us/call with pure-XOR placement, probe-verified).

x@Wk for all gates rides in the same PSUM accumulation as two extra
k-tiles (stationary = per-step x^T slice), so there is no zx precompute
and no identity-seed matmuls. Next step's x-tile matmuls are emitted
right after this step's h-tiles to shorten PE idle gaps (HAM warmth).
"""

import numpy as np

VOCAB = 2048
EMB = 256
MAXLEN = 8
BATCH = 16
NCORES = 8
NSHARD = 4
U = VOCAB // NSHARD          # 512 hidden units per core (shard = rank & 3)
PCOLS = 4 * 32 + 1           # pay cols: hT image 128 + sums col = 129

_CACHE = {}


def _host_tokens(input_point):
    """token[b,t] = argmax_k((k/V <= v) & (v <= (k+1)/V)), first-true wins.
    Exact: v is fp32, k/V dyadic; replicate in float64."""
    v = input_point[:, :MAXLEN].astype(np.float64)
    u = v * VOCAB
    j = np.floor(u)
    exact = (u == j) & (j > 0)
    tok = np.where(exact, j - 1, j)
    return np.clip(tok, 0, VOCAB - 1).astype(np.int32)


def _build_nc(masked, b_nonzero):
    import concourse.mybir as mybir
    import concourse.tile as tile
    from concourse import bacc

    f32 = mybir.dt.float32
    bf16 = mybir.dt.bfloat16
    AF = mybir.ActivationFunctionType
    OP = mybir.AluOpType

    nc = bacc.Bacc(None, target_bir_lowering=False, debug=False)

    # wblk rows: 16 h-ktiles (XOR-permuted Wr/2) + 2 x-ktiles (Wk) + bias
    wblk = nc.dram_tensor("wblk", [2305, 4 * U], bf16, kind="ExternalInput")
    xt_ext = nc.dram_tensor("xt", [128, 2 * MAXLEN * BATCH], bf16, kind="ExternalInput")
    mask_ext = nc.dram_tensor("masks", [BATCH, MAXLEN], f32, kind="ExternalInput")
    out_ext = nc.dram_tensor("out", [BATCH, MAXLEN * U], f32, kind="ExternalOutput")

    from contextlib import ExitStack

    with ExitStack() as _st:
        rsem = _st.enter_context(nc.semaphore("rsem"))
        lsem = _st.enter_context(nc.semaphore("lsem"))
        tc = _st.enter_context(tile.TileContext(nc))
        with (
            tc.tile_pool(name="const", bufs=1) as cp,
            tc.tile_pool(name="state", bufs=2) as sp,
            tc.tile_pool(name="work", bufs=3) as wk,
            tc.tile_pool(name="zps", bufs=2, space="PSUM") as psz,
        ):
            with tc.tile_critical():
                nc.gpsimd.sem_clear(rsem)
                nc.gpsimd.sem_clear(lsem)
                nc.gpsimd.bir_kernel_barrier_wait([list(range(NCORES))])

            xt_sb = cp.tile([128, 2 * MAXLEN * BATCH], bf16)
            nc.sync.dma_start(xt_sb[:], xt_ext[:])
            mask_sb = cp.tile([BATCH, MAXLEN], f32)
            nc.sync.dma_start(mask_sb[:], mask_ext[:])

            W = 4 * U  # 2048 cols per core: [i|f|g|o] of shard b
            wsb = cp.tile([128, 18 * W], bf16)
            dma_engs = [nc.sync, nc.scalar, nc.gpsimd]
            for i, j in enumerate((16, 17)):  # x-tiles first
                dma_engs[i % 3].dma_start(
                    wsb[:, W * j:W * (j + 1)], wblk[128 * j:128 * (j + 1), :]
                )
            if b_nonzero:
                wb = cp.tile([1, W], bf16)
                onesrow = cp.tile([1, 16], bf16)
                nc.gpsimd.memset(onesrow[:], 1.0)
                nc.sync.dma_start(wb[:], wblk[2304:2305, :])
            # h-tile loads are issued AFTER step 0's sends (see loop below) so
            # the 9.4MB weight stream doesn't queue ahead of the first gather
            # on the shared DMA rings.
            def load_h_tiles():
                for j in range(16):
                    dma_engs[j % 3].dma_start(
                        wsb[:, W * j:W * (j + 1)], wblk[128 * j:128 * (j + 1), :]
                    )

            exp_sb = cp.tile([BATCH, MAXLEN * U], f32)
            out_sb = cp.tile([BATCH, MAXLEN * U], f32)

            # bf16 h staging for DVE transpose: rows 16:32 zeroed once
            hb_slots = []
            for i in range(2):
                hb = cp.tile([32, U], bf16, tag=f"hb{i}")
                nc.vector.memset(hb[:], 0.0)
                hb_slots.append(hb)

            # pay = [hT image (128 x 128, batch padded to 32) | sums col]
            pay_slots = []
            hT_slots = []
            for i in range(2):
                pay = cp.tile([128, PCOLS], bf16, tag=f"pay{i}")
                nc.vector.memset(pay[:], 0.0)
                pay_slots.append(pay)
                hTs = cp.tile([128, 4 * PCOLS], bf16, tag=f"hTs{i}")
                hT_slots.append(hTs)
            rd_same = {d: [(0, d)] * 8 for d in (1, 2, 3)}

            if masked:
                h_prev = sp.tile([BATCH, U], f32, tag="h")
                nc.vector.memset(h_prev[:], 0.0)
            c_prev = sp.tile([BATCH, U], f32, tag="c")
            nc.vector.memset(c_prev[:], 0.0)

            def stat_ap(t, d, m):
                """stationary [128,16] for k-tile (d,m) of step t's h."""
                if d == 0:
                    return pay_slots[t % 2][:, 32 * m:32 * m + 16]
                return hT_slots[t % 2][:, PCOLS * d + 32 * m:PCOLS * d + 32 * m + 16]

            def softmax_out(t):
                """Denominator from own acc + 3 gathered sums; scale + out."""
                hTs = hT_slots[t % 2]
                sums_ap = hTs[0:16, :].rearrange(
                    "b (d c) -> b d c", c=PCOLS
                )[:, 1:4, PCOLS - 1]
                tot3 = wk.tile([BATCH, 1], f32, tag="tot")
                tot = wk.tile([BATCH, 1], f32, tag="tot4")
                rec = wk.tile([BATCH, 1], f32, tag="rec")
                nc.vector.tensor_reduce(
                    tot3[:], sums_ap, mybir.AxisListType.X, OP.add
                )
                nc.vector.tensor_tensor(tot[:], tot3[:], accs[t][:], OP.add)
                nc.vector.reciprocal(rec[:], tot[:])
                nc.vector.tensor_scalar_mul(
                    out_sb[:, U * t:U * (t + 1)], exp_sb[:, U * t:U * (t + 1)],
                    rec[:],
                )
                nc.gpsimd.dma_start(
                    out_ext[:, U * t:U * (t + 1)], out_sb[:, U * t:U * (t + 1)]
                )

            accs = {}
            z_tiles = {}

            def x_rounds(t):
                """Seed step t's 4 z banks with x@Wk (+ b): 2 k-tiles."""
                zt = [
                    psz.tile([BATCH, 512], f32, tag=f"z{bk}", name=f"z{bk}")
                    for bk in range(4)
                ]
                z_tiles[t] = zt
                # step 0 has no h-rounds: its group closes here
                last = (t == 0)
                for kx in (16, 17):
                    xs = xt_sb[:, 16 * ((kx - 16) * MAXLEN + t):
                               16 * ((kx - 16) * MAXLEN + t) + 16]
                    for bk in range(4):
                        nc.tensor.matmul(
                            zt[bk][:], xs, wsb[:, W * kx + 512 * bk:W * kx + 512 * (bk + 1)],
                            start=(kx == 16),
                            stop=(last and kx == 17 and not b_nonzero),
                        )
                if b_nonzero:
                    for bk in range(4):
                        nc.tensor.matmul(
                            zt[bk][:], onesrow[:], wb[:, 512 * bk:512 * (bk + 1)],
                            start=False, stop=last,
                        )

            x_rounds(0)
            for t in range(MAXLEN):
                zt = z_tiles[t]
                # ---- h-rounds: slice-major, pipelined on arrivals ----
                # consumes step t-1's h: pay/hT slots of parity (t-1) % 2
                if t > 0:
                    hTs = hT_slots[(t - 1) % 2]
                    # two-phase arrival pipelining: slices {0,1} after the
                    # first peer transfer lands, {2,3} after all three
                    with tc.tile_critical():
                        nc.vector.wait_ge(rsem, 48 * (t - 1) + 16)
                        nc.vector.tensor_copy(hTs[0:1, 0:1], hTs[0:1, 0:1])
                    for d in range(2):
                        for m in range(4):
                            kt = 4 * d + m
                            st = stat_ap(t - 1, d, m)
                            for bk in range(4):
                                nc.tensor.matmul(
                                    zt[bk][:], st,
                                    wsb[:, W * kt + 512 * bk:W * kt + 512 * (bk + 1)],
                                    start=False, stop=False,
                                )
                    with tc.tile_critical():
                        nc.vector.wait_ge(rsem, 48 * t)
                        nc.vector.tensor_copy(hTs[0:1, 0:1], hTs[0:1, 0:1])
                    for d in range(2, 4):
                        for m in range(4):
                            kt = 4 * d + m
                            st = stat_ap(t - 1, d, m)
                            for bk in range(4):
                                nc.tensor.matmul(
                                    zt[bk][:], st,
                                    wsb[:, W * kt + 512 * bk:W * kt + 512 * (bk + 1)],
                                    start=False, stop=(kt == 15),
                                )
                    # own sends of step t-2 drained (pay slot reuse)
                    with tc.tile_critical():
                        nc.vector.wait_ge(lsem, 48 * (t - 1))
                        nc.vector.tensor_copy(
                            pay_slots[t % 2][0:1, 0:1], pay_slots[t % 2][0:1, 0:1]
                        )
                    softmax_out(t - 1)
                if t + 1 < MAXLEN:
                    x_rounds(t + 1)

                # ---- gates, tanh-only: sig(x) = (tanh(x/2)+1)/2 ----
                t_i = wk.tile([BATCH, U], f32, tag="ti")
                t_f = wk.tile([BATCH, U], f32, tag="tf")
                tg = wk.tile([BATCH, U], f32, tag="tg")
                t_o = wk.tile([BATCH, U], f32, tag="to")
                nc.scalar.activation(t_i[:], zt[0][:], AF.Tanh, scale=0.5)
                nc.scalar.activation(t_f[:], zt[1][:], AF.Tanh, scale=0.5)
                nc.scalar.activation(tg[:], zt[2][:], AF.Tanh)
                nc.scalar.activation(t_o[:], zt[3][:], AF.Tanh, scale=0.5)

                hbt = hb_slots[t % 2]
                acc = wk.tile([BATCH, 1], f32, tag=f"acc{t}")
                accs[t] = acc
                c_next = sp.tile([BATCH, U], f32, tag="c")
                t1 = wk.tile([BATCH, U], f32, tag="t1")
                t2 = wk.tile([BATCH, U], f32, tag="t2")
                if not masked:
                    # doubled state: C = 2c, hb = 2h; Wr pre-halved on host
                    nc.vector.scalar_tensor_tensor(
                        t2[:], t_i[:], 1.0, tg[:], OP.add, OP.mult
                    )
                    nc.vector.scalar_tensor_tensor(
                        t1[:], t_f[:], 1.0, c_prev[:], OP.add, OP.mult
                    )
                    nc.vector.scalar_tensor_tensor(
                        c_next[:], t1[:], 0.5, t2[:], OP.mult, OP.add
                    )
                    tc_ = wk.tile([BATCH, U], f32, tag="tc")
                    nc.scalar.activation(tc_[:], c_next[:], AF.Tanh, scale=0.5)
                    nc.vector.scalar_tensor_tensor(
                        hbt[0:16, :], t_o[:], 1.0, tc_[:], OP.add, OP.mult
                    )
                    nc.scalar.activation(
                        exp_sb[:, U * t:U * (t + 1)], hbt[0:16, :], AF.Exp,
                        scale=0.5, accum_out=acc[:],
                    )
                else:
                    m_t = mask_sb[:, t:t + 1]
                    cn = wk.tile([BATCH, U], f32, tag="cn")
                    dm = wk.tile([BATCH, U], f32, tag="dm")
                    nc.vector.scalar_tensor_tensor(
                        t2[:], t_i[:], 1.0, tg[:], OP.add, OP.mult
                    )
                    nc.vector.scalar_tensor_tensor(
                        t1[:], t_f[:], 1.0, c_prev[:], OP.add, OP.mult
                    )
                    nc.vector.scalar_tensor_tensor(
                        cn[:], t1[:], 0.5, t2[:], OP.mult, OP.add
                    )
                    nc.vector.tensor_tensor(dm[:], cn[:], c_prev[:], OP.subtract)
                    nc.vector.scalar_tensor_tensor(
                        c_next[:], dm[:], m_t, c_prev[:], OP.mult, OP.add
                    )
                    tc_ = wk.tile([BATCH, U], f32, tag="tc")
                    nc.scalar.activation(tc_[:], c_next[:], AF.Tanh, scale=0.5)
                    hn = wk.tile([BATCH, U], f32, tag="hn")
                    dh = wk.tile([BATCH, U], f32, tag="dh")
                    h_next = sp.tile([BATCH, U], f32, tag="h")
                    nc.vector.scalar_tensor_tensor(
                        hn[:], t_o[:], 1.0, tc_[:], OP.add, OP.mult
                    )
                    nc.vector.tensor_tensor(dh[:], hn[:], h_prev[:], OP.subtract)
                    nc.vector.scalar_tensor_tensor(
                        h_next[:], dh[:], m_t, h_prev[:], OP.mult, OP.add
                    )
                    nc.vector.tensor_copy(hbt[0:16, :], h_next[:])
                    nc.scalar.activation(
                        exp_sb[:, U * t:U * (t + 1)], h_next[:], AF.Exp,
                        scale=0.5, accum_out=acc[:],
                    )
                    h_prev = h_next

                # ---- transpose 2h into pay hT image + sums col ----
                pay = pay_slots[t % 2]
                for q in range(16):
                    ph, m = q % 4, q // 4
                    nc.vector.transpose(
                        pay[32 * ph:32 * ph + 32, 32 * m:32 * m + 32],
                        hbt[:, 32 * q:32 * q + 32],
                    )
                nc.vector.tensor_copy(pay[0:16, PCOLS - 1:PCOLS], acc[:])

                # ---- 3 same-die repeated-dest broadcasts ----
                for d in (1, 2, 3):
                    nc.gpsimd.remote_dma_broadcast(
                        hT_slots[t % 2][:, PCOLS * d:PCOLS * (d + 1)],
                        pay[:, 0:PCOLS],
                        rsem,
                        lsem,
                        rdests=rd_same[d],
                    )
                nc.gpsimd.trigger_dma(count=None)
                if t == 0:
                    load_h_tiles()

                c_prev = c_next

            # ---- last step's softmax needs the final gather ----
            hTs = hT_slots[(MAXLEN - 1) % 2]
            with tc.tile_critical():
                nc.vector.wait_ge(rsem, 48 * MAXLEN)
                nc.vector.tensor_copy(hTs[0:1, 0:1], hTs[0:1, 0:1])
            softmax_out(MAXLEN - 1)

    nc.compile()
    return nc


def _get_nc(masked=False, b_nonzero=False):
    key = (masked, b_nonzero)
    if key not in _CACHE:
        _CACHE[key] = _build_nc(masked, b_nonzero)
    return _CACHE[key]


def _host_prep(input_point, E, Wk, Wr, b):
    import ml_dtypes
    bf = ml_dtypes.bfloat16

    ip = np.ascontiguousarray(np.asarray(input_point, dtype=np.float32))
    E = np.asarray(E, dtype=np.float32)
    Wk = np.asarray(Wk, dtype=np.float32)
    Wr = np.asarray(Wr, dtype=np.float32)
    b = np.asarray(b, dtype=np.float32)

    tokens = _host_tokens(ip)                                # (B, T)
    masks = (tokens != 0).astype(np.float32)                 # (B, T)
    X = E[tokens]                                            # (B, T, EMB)

    # xt[p, 16*(kt*T + t) + bb] = X[bb, t, 128*kt + p]
    xt = np.transpose(X.reshape(BATCH, MAXLEN, 2, 128), (2, 3, 1, 0))  # (2,128,T,B)
    xt = np.ascontiguousarray(xt.reshape(2, 128, MAXLEN * BATCH))
    xt = np.concatenate([xt[0], xt[1]], axis=1).astype(bf)   # (128, 2*T*B)

    Wr2 = Wr * 0.5
    tail = np.vstack([Wk, b[None, :]]).astype(np.float32)    # (257, 4V)
    in_maps = []
    for r in range(NCORES):
        sh = r & 3
        cols = np.concatenate(
            [np.arange(g * VOCAB + sh * U, g * VOCAB + (sh + 1) * U)
             for g in range(4)]
        )
        rows = np.concatenate(
            [np.arange(U * (sh ^ d), U * ((sh ^ d) + 1)) for d in range(NSHARD)]
        )
        W_aug = np.vstack([Wr2[rows], tail])                 # (2305, 4V)
        in_maps.append({
            "wblk": np.ascontiguousarray(W_aug[:, cols]).astype(bf),
            "xt": xt,
            "masks": np.ascontiguousarray(masks),
        })
    flags = (bool((masks != 1.0).any()), bool(np.any(b != 0.0)))
    return in_maps, flags


def _assemble(results):
    out = np.empty((BATCH, MAXLEN, VOCAB), dtype=np.float32)
    for r in range(NSHARD):
        blk = results[r]["out"].reshape(BATCH, MAXLEN, U)
        out[:, :, r * U:(r + 1) * U] = blk
    return out


def kernel(input_point, E, Wk, Wr, b):
    from concourse.bass_utils import run_bass_kernel_spmd

    in_maps, flags = _host_prep(input_point, E, Wk, Wr, b)
    nc = _get_nc(*flags)
    res = run_bass_kernel_spmd(nc, in_maps, list(range(NCORES)))
    return _assemble(res.results)


# revision 18
# speedup vs baseline: 1.7949x; 1.0078x over previous
"""Trainium2 Bass kernel for nn_DAriEL_Decoder_Cell_1_88064009437441.

Reference structure: tokens depend only on input_point (the cell resets
one_softmax/unfolding each step), so the device work is one 8-step LSTM
(2048 units, gate dim 8192) + per-step softmax of h_t.

Distribution: 4-way tensor-parallel over the hidden/gate dim (U=512 per
shard), with each shard REPLICATED on both dies (core r holds shard r&3).
Every core therefore gathers the other 3 h-slices from SAME-DIE peers
(r^1, r^2, r^3) via repeated-dest relative remote_dma_broadcast — all 16
DMA lanes carry real payload (the per-peer single-dest broadcast of the
old kernel serialized ~42us/step of dummy-lane descriptors; this one does
~2.3us/call with pure-XOR placement, probe-verified).

x@Wk for all gates rides in the same PSUM accumulation as two extra
k-tiles (stationary = per-step x^T slice), so there is no zx precompute
and no identity-seed matmuls. Next step's x-tile matmuls are emitted
right after this step's h-tiles to shorten PE idle gaps (HAM warmth).
"""

import numpy as np

VOCAB = 2048
EMB = 256
MAXLEN = 8
BATCH = 16
NCORES = 8
NSHARD = 4
U = VOCAB // NSHARD          # 512 hidden units per core (shard = rank & 3)
PCOLS = 4 * 32 + 1           # pay cols: hT image 128 + sums col = 129

_CACHE = {}


def _host_tokens(input_point):
    """token[b,t] = argmax_k((k/V <= v) & (v <= (k+1)/V)), first-true wins.
    Exact: v is fp32, k/V dyadic; replicate in float64."""
    v = input_point[:, :MAXLEN].astype(np.float64)
    u = v * VOCAB
    j = np.floor(u)
    exact = (u == j) & (j > 0)
    tok = np.where(exact, j - 1, j)
    return np.clip(tok, 0, VOCAB - 1).astype(np.int32)


def _build_nc(masked, b_nonzero):
    import concourse.mybir as mybir
    import concourse.tile as tile
    from concourse import bacc

    f32 = mybir.dt.float32
    bf16 = mybir.dt.bfloat16
    AF = mybir.ActivationFunctionType
    OP = mybir.AluOpType

    nc = bacc.Bacc(None, target_bir_lowering=False, debug=False)

    # wblk rows: 16 h-ktiles (XOR-permuted Wr/2) + 2 x-ktiles (Wk) + bias
    wblk = nc.dram_tensor("wblk", [2305, 4 * U], bf16, kind="ExternalInput")
    xt_ext = nc.dram_tensor("xt", [128, 2 * MAXLEN * BATCH], bf16, kind="ExternalInput")
    mask_ext = nc.dram_tensor("masks", [BATCH, MAXLEN], f32, kind="ExternalInput")
    out_ext = nc.dram_tensor("out", [BATCH, MAXLEN * U], f32, kind="ExternalOutput")

    from contextlib import ExitStack

    with ExitStack() as _st:
        rsem = _st.enter_context(nc.semaphore("rsem"))
        lsem = _st.enter_context(nc.semaphore("lsem"))
        wsem = _st.enter_context(nc.semaphore("wsem"))
        tc = _st.enter_context(tile.TileContext(nc))
        with (
            tc.tile_pool(name="const", bufs=1) as cp,
            tc.tile_pool(name="state", bufs=2) as sp,
            tc.tile_pool(name="work", bufs=3) as wk,
            tc.tile_pool(name="zps", bufs=2, space="PSUM") as psz,
        ):
            with tc.tile_critical():
                nc.gpsimd.sem_clear(rsem)
                nc.gpsimd.sem_clear(lsem)
                nc.gpsimd.bir_kernel_barrier_wait([list(range(NCORES))])
            nc.gpsimd.remote_sem_update_broadcast(
                wsem, wsem, rdests=[(0, 0)] * 8
            )
            nc.gpsimd.trigger_dma(count=None)

            xt_sb = cp.tile([128, 2 * MAXLEN * BATCH], bf16)
            nc.sync.dma_start(xt_sb[:], xt_ext[:])
            mask_sb = cp.tile([BATCH, MAXLEN], f32)
            nc.sync.dma_start(mask_sb[:], mask_ext[:])

            W = 4 * U  # 2048 cols per core: [i|f|g|o] of shard b
            wsb = cp.tile([128, 18 * W], bf16)
            dma_engs = [nc.sync, nc.scalar, nc.sync]
            for i, j in enumerate((16, 17)):  # x-tiles first
                dma_engs[i % 3].dma_start(
                    wsb[:, W * j:W * (j + 1)], wblk[128 * j:128 * (j + 1), :]
                )
            if b_nonzero:
                wb = cp.tile([1, W], bf16)
                onesrow = cp.tile([1, 16], bf16)
                nc.gpsimd.memset(onesrow[:], 1.0)
                nc.sync.dma_start(wb[:], wblk[2304:2305, :])
            # h-tile loads are issued AFTER step 0's sends (see loop below) so
            # the 9.4MB weight stream doesn't queue ahead of the first gather
            # on the shared DMA rings.
            def load_h_tiles():
                for j in range(16):
                    dma_engs[j % 3].dma_start(
                        wsb[:, W * j:W * (j + 1)], wblk[128 * j:128 * (j + 1), :]
                    )

            exp_sb = cp.tile([BATCH, MAXLEN * U], f32)
            out_sb = cp.tile([BATCH, MAXLEN * U], f32)

            # bf16 h staging for DVE transpose: rows 16:32 zeroed once
            hb_slots = []
            for i in range(2):
                hb = cp.tile([32, U], bf16, tag=f"hb{i}")
                nc.vector.memset(hb[:], 0.0)
                hb_slots.append(hb)

            # pay = [hT image (128 x 128, batch padded to 32) | sums col]
            pay_slots = []
            hT_slots = []
            for i in range(2):
                pay = cp.tile([128, PCOLS], bf16, tag=f"pay{i}")
                nc.vector.memset(pay[:], 0.0)
                pay_slots.append(pay)
                hTs = cp.tile([128, 4 * PCOLS], bf16, tag=f"hTs{i}")
                hT_slots.append(hTs)
            rd_same = {d: [(0, d)] * 8 for d in (1, 2, 3)}

            if masked:
                h_prev = sp.tile([BATCH, U], f32, tag="h")
                nc.vector.memset(h_prev[:], 0.0)
            c_prev = sp.tile([BATCH, U], f32, tag="c")
            nc.vector.memset(c_prev[:], 0.0)

            def stat_ap(t, d, m):
                """stationary [128,16] for k-tile (d,m) of step t's h."""
                if d == 0:
                    return pay_slots[t % 2][:, 32 * m:32 * m + 16]
                return hT_slots[t % 2][:, PCOLS * d + 32 * m:PCOLS * d + 32 * m + 16]

            def softmax_out(t):
                """Denominator from own acc + 3 gathered sums; scale + out."""
                hTs = hT_slots[t % 2]
                sums_ap = hTs[0:16, :].rearrange(
                    "b (d c) -> b d c", c=PCOLS
                )[:, 1:4, PCOLS - 1]
                tot3 = wk.tile([BATCH, 1], f32, tag="tot")
                tot = wk.tile([BATCH, 1], f32, tag="tot4")
                rec = wk.tile([BATCH, 1], f32, tag="rec")
                nc.vector.tensor_reduce(
                    tot3[:], sums_ap, mybir.AxisListType.X, OP.add
                )
                nc.vector.tensor_tensor(tot[:], tot3[:], accs[t][:], OP.add)
                nc.vector.reciprocal(rec[:], tot[:])
                nc.vector.tensor_scalar_mul(
                    out_sb[:, U * t:U * (t + 1)], exp_sb[:, U * t:U * (t + 1)],
                    rec[:],
                )
                nc.sync.dma_start(
                    out_ext[:, U * t:U * (t + 1)], out_sb[:, U * t:U * (t + 1)]
                )

            accs = {}
            z_tiles = {}

            def x_rounds(t):
                """Seed step t's 4 z banks with x@Wk (+ b): 2 k-tiles."""
                zt = [
                    psz.tile([BATCH, 512], f32, tag=f"z{bk}", name=f"z{bk}")
                    for bk in range(4)
                ]
                z_tiles[t] = zt
                # step 0 has no h-rounds: its group closes here
                last = (t == 0)
                for kx in (16, 17):
                    xs = xt_sb[:, 16 * ((kx - 16) * MAXLEN + t):
                               16 * ((kx - 16) * MAXLEN + t) + 16]
                    for bk in range(4):
                        nc.tensor.matmul(
                            zt[bk][:], xs, wsb[:, W * kx + 512 * bk:W * kx + 512 * (bk + 1)],
                            start=(kx == 16),
                            stop=(last and kx == 17 and not b_nonzero),
                        )
                if b_nonzero:
                    for bk in range(4):
                        nc.tensor.matmul(
                            zt[bk][:], onesrow[:], wb[:, 512 * bk:512 * (bk + 1)],
                            start=False, stop=last,
                        )

            x_rounds(0)
            for t in range(MAXLEN):
                zt = z_tiles[t]
                # ---- h-rounds: slice-major, pipelined on arrivals ----
                # consumes step t-1's h: pay/hT slots of parity (t-1) % 2
                if t > 0:
                    hTs = hT_slots[(t - 1) % 2]
                    # own slice needs no wait; slice 1 after the first
                    # peer transfer lands, {2,3} after all three
                    for m in range(4):
                        st = stat_ap(t - 1, 0, m)
                        for bk in range(4):
                            nc.tensor.matmul(
                                zt[bk][:], st,
                                wsb[:, W * m + 512 * bk:W * m + 512 * (bk + 1)],
                                start=False, stop=False,
                            )
                    with tc.tile_critical():
                        nc.vector.wait_ge(rsem, 48 * (t - 1) + 16)
                        nc.vector.tensor_copy(hTs[0:1, 0:1], hTs[0:1, 0:1])
                    for d in range(1, 2):
                        for m in range(4):
                            kt = 4 * d + m
                            st = stat_ap(t - 1, d, m)
                            for bk in range(4):
                                nc.tensor.matmul(
                                    zt[bk][:], st,
                                    wsb[:, W * kt + 512 * bk:W * kt + 512 * (bk + 1)],
                                    start=False, stop=False,
                                )
                    with tc.tile_critical():
                        nc.vector.wait_ge(rsem, 48 * (t - 1) + 32)
                        nc.vector.tensor_copy(hTs[0:1, 0:1], hTs[0:1, 0:1])
                    for d in range(2, 3):
                        for m in range(4):
                            kt = 4 * d + m
                            st = stat_ap(t - 1, d, m)
                            for bk in range(4):
                                nc.tensor.matmul(
                                    zt[bk][:], st,
                                    wsb[:, W * kt + 512 * bk:W * kt + 512 * (bk + 1)],
                                    start=False, stop=False,
                                )
                    with tc.tile_critical():
                        nc.vector.wait_ge(rsem, 48 * t)
                        nc.vector.tensor_copy(hTs[0:1, 0:1], hTs[0:1, 0:1])
                        nc.vector.wait_ge(lsem, 48 * (t - 1))
                        nc.vector.tensor_copy(
                            pay_slots[t % 2][0:1, 0:1], pay_slots[t % 2][0:1, 0:1]
                        )
                    for d in range(3, 4):
                        for m in range(4):
                            kt = 4 * d + m
                            st = stat_ap(t - 1, d, m)
                            for bk in range(4):
                                nc.tensor.matmul(
                                    zt[bk][:], st,
                                    wsb[:, W * kt + 512 * bk:W * kt + 512 * (bk + 1)],
                                    start=False, stop=(kt == 15),
                                )
                    softmax_out(t - 1)
                if t + 1 < MAXLEN:
                    x_rounds(t + 1)

                # ---- gates, tanh-only: sig(x) = (tanh(x/2)+1)/2 ----
                t_i = wk.tile([BATCH, U], f32, tag="ti")
                t_f = wk.tile([BATCH, U], f32, tag="tf")
                tg = wk.tile([BATCH, U], f32, tag="tg")
                t_o = wk.tile([BATCH, U], f32, tag="to")
                nc.scalar.activation(t_i[:], zt[0][:], AF.Tanh, scale=0.5)
                nc.scalar.activation(tg[:], zt[2][:], AF.Tanh)
                nc.scalar.activation(t_f[:], zt[1][:], AF.Tanh, scale=0.5)
                nc.scalar.activation(t_o[:], zt[3][:], AF.Tanh, scale=0.5)

                hbt = hb_slots[t % 2]
                acc = wk.tile([BATCH, 1], f32, tag=f"acc{t}")
                accs[t] = acc
                c_next = sp.tile([BATCH, U], f32, tag="c")
                t1 = wk.tile([BATCH, U], f32, tag="t1")
                t2 = wk.tile([BATCH, U], f32, tag="t2")
                if not masked:
                    # doubled state: C = 2c, hb = 2h; Wr pre-halved on host
                    nc.vector.scalar_tensor_tensor(
                        t2[:], t_i[:], 1.0, tg[:], OP.add, OP.mult
                    )
                    nc.vector.scalar_tensor_tensor(
                        t1[:], t_f[:], 1.0, c_prev[:], OP.add, OP.mult
                    )
                    nc.vector.scalar_tensor_tensor(
                        c_next[:], t1[:], 0.5, t2[:], OP.mult, OP.add
                    )
                    tc_ = wk.tile([BATCH, U], f32, tag="tc")
                    nc.scalar.activation(tc_[:], c_next[:], AF.Tanh, scale=0.5)
                    nc.vector.scalar_tensor_tensor(
                        hbt[0:16, :], t_o[:], 1.0, tc_[:], OP.add, OP.mult
                    )
                    nc.scalar.activation(
                        exp_sb[:, U * t:U * (t + 1)], hbt[0:16, :], AF.Exp,
                        scale=0.5, accum_out=acc[:],
                    )
                else:
                    m_t = mask_sb[:, t:t + 1]
                    cn = wk.tile([BATCH, U], f32, tag="cn")
                    dm = wk.tile([BATCH, U], f32, tag="dm")
                    nc.vector.scalar_tensor_tensor(
                        t2[:], t_i[:], 1.0, tg[:], OP.add, OP.mult
                    )
                    nc.vector.scalar_tensor_tensor(
                        t1[:], t_f[:], 1.0, c_prev[:], OP.add, OP.mult
                    )
                    nc.vector.scalar_tensor_tensor(
                        cn[:], t1[:], 0.5, t2[:], OP.mult, OP.add
                    )
                    nc.vector.tensor_tensor(dm[:], cn[:], c_prev[:], OP.subtract)
                    nc.vector.scalar_tensor_tensor(
                        c_next[:], dm[:], m_t, c_prev[:], OP.mult, OP.add
                    )
                    tc_ = wk.tile([BATCH, U], f32, tag="tc")
                    nc.scalar.activation(tc_[:], c_next[:], AF.Tanh, scale=0.5)
                    hn = wk.tile([BATCH, U], f32, tag="hn")
                    dh = wk.tile([BATCH, U], f32, tag="dh")
                    h_next = sp.tile([BATCH, U], f32, tag="h")
                    nc.vector.scalar_tensor_tensor(
                        hn[:], t_o[:], 1.0, tc_[:], OP.add, OP.mult
                    )
                    nc.vector.tensor_tensor(dh[:], hn[:], h_prev[:], OP.subtract)
                    nc.vector.scalar_tensor_tensor(
                        h_next[:], dh[:], m_t, h_prev[:], OP.mult, OP.add
                    )
                    nc.vector.tensor_copy(hbt[0:16, :], h_next[:])
                    nc.scalar.activation(
                        exp_sb[:, U * t:U * (t + 1)], h_next[:], AF.Exp,
                        scale=0.5, accum_out=acc[:],
                    )
                    h_prev = h_next

                # ---- transpose 2h into pay hT image + sums col ----
                pay = pay_slots[t % 2]
                for q in range(16):
                    ph, m = q % 4, q // 4
                    nc.vector.transpose(
                        pay[32 * ph:32 * ph + 32, 32 * m:32 * m + 32],
                        hbt[:, 32 * q:32 * q + 32],
                    )
                nc.vector.tensor_copy(pay[0:16, PCOLS - 1:PCOLS], acc[:])

                # ---- 3 same-die repeated-dest broadcasts ----
                for d in (1, 2, 3):
                    nc.gpsimd.remote_dma_broadcast(
                        hT_slots[t % 2][:, PCOLS * d:PCOLS * (d + 1)],
                        pay[:, 0:PCOLS],
                        rsem,
                        lsem,
                        rdests=rd_same[d],
                    )
                nc.gpsimd.trigger_dma(count=None)
                if t == 0:
                    load_h_tiles()

                c_prev = c_next

            # ---- last step's softmax needs the final gather ----
            hTs = hT_slots[(MAXLEN - 1) % 2]
            with tc.tile_critical():
                nc.vector.wait_ge(rsem, 48 * MAXLEN)
                nc.vector.tensor_copy(hTs[0:1, 0:1], hTs[0:1, 0:1])
            softmax_out(MAXLEN - 1)

    nc.compile()
    return nc


def _get_nc(masked=False, b_nonzero=False):
    key = (masked, b_nonzero)
    if key not in _CACHE:
        _CACHE[key] = _build_nc(masked, b_nonzero)
    return _CACHE[key]


def _host_prep(input_point, E, Wk, Wr, b):
    import ml_dtypes
    bf = ml_dtypes.bfloat16

    ip = np.ascontiguousarray(np.asarray(input_point, dtype=np.float32))
    E = np.asarray(E, dtype=np.float32)
    Wk = np.asarray(Wk, dtype=np.float32)
    Wr = np.asarray(Wr, dtype=np.float32)
    b = np.asarray(b, dtype=np.float32)

    tokens = _host_tokens(ip)                                # (B, T)
    masks = (tokens != 0).astype(np.float32)                 # (B, T)
    X = E[tokens]                                            # (B, T, EMB)

    # xt[p, 16*(kt*T + t) + bb] = X[bb, t, 128*kt + p]
    xt = np.transpose(X.reshape(BATCH, MAXLEN, 2, 128), (2, 3, 1, 0))  # (2,128,T,B)
    xt = np.ascontiguousarray(xt.reshape(2, 128, MAXLEN * BATCH))
    xt = np.concatenate([xt[0], xt[1]], axis=1).astype(bf)   # (128, 2*T*B)

    Wr2 = Wr * 0.5
    tail = np.vstack([Wk, b[None, :]]).astype(np.float32)    # (257, 4V)
    in_maps = []
    for r in range(NCORES):
        sh = r & 3
        cols = np.concatenate(
            [np.arange(g * VOCAB + sh * U, g * VOCAB + (sh + 1) * U)
             for g in range(4)]
        )
        rows = np.concatenate(
            [np.arange(U * (sh ^ d), U * ((sh ^ d) + 1)) for d in range(NSHARD)]
        )
        W_aug = np.vstack([Wr2[rows], tail])                 # (2305, 4V)
        in_maps.append({
            "wblk": np.ascontiguousarray(W_aug[:, cols]).astype(bf),
            "xt": xt,
            "masks": np.ascontiguousarray(masks),
        })
    flags = (bool((masks != 1.0).any()), bool(np.any(b != 0.0)))
    return in_maps, flags


def _assemble(results):
    out = np.empty((BATCH, MAXLEN, VOCAB), dtype=np.float32)
    for r in range(NSHARD):
        blk = results[r]["out"].reshape(BATCH, MAXLEN, U)
        out[:, :, r * U:(r + 1) * U] = blk
    return out


def kernel(input_point, E, Wk, Wr, b):
    from concourse.bass_utils import run_bass_kernel_spmd

    in_maps, flags = _host_prep(input_point, E, Wk, Wr, b)
    nc = _get_nc(*flags)
    res = run_bass_kernel_spmd(nc, in_maps, list(range(NCORES)))
    return _assemble(res.results)
